# revision 46
# baseline (speedup 1.0000x reference)
"""Multi-head GQA attention (B=2, S=2048, D=2048, H=16, KVH=4) on 8 TRN2
NeuronCores.

Sharding: core i = (b, g) with b = i // 4 (batch), g = i % 4 (sequence
chunk of 512 queries). Each core computes Q for its 512 queries over all
16 heads, K/V for its own 512 sequence positions, AllGathers K/V within
its 4-core batch group, then runs full attention + output projection for
its query chunk. Host concatenates the 8 [512, 2048] chunks.

Layout strategy (no on-chip transposes):
 - host passes x transposed per chunk (xT [D, 512]) so projections
   computed as w.T @ xT yield QT/KT with head-dim on partitions —
   exactly the operand layout attention needs.
 - wq/wk columns permuted per head (even dims first, odd second) so RoPE
   halves are contiguous partition ranges [0:64)/[64:128). Scores are
   permutation-invariant since q and k are permuted identically.
 - scores computed transposed (ST[k, q] = KT.T @ QT), exp'd on ScalarE
   straight out of PSUM (scale=1/sqrt(HD) folded in, no max-subtraction:
   scores are O(10) so f32 exp is safe), giving probs in the [k, q]
   layout the AV matmul wants as its moving operand.
 - softmax denominator from an all-ones [k,128] stationary matmul: the
   output is the denominator replicated across all 128 partitions, so
   normalization is reciprocal + elementwise multiply, no broadcast.
 - weights are host-pretiled so every DMA is a contiguous block.
"""

import numpy as np
import ml_dtypes

B, S, D = 2, 2048, 2048
H, KVH = 16, 4
HD = D // H            # 128
R = H // KVH           # 4 (GQA repeat)
NCORES = 8
G = 4                  # cores per batch group = seq chunks
SQ = S // G            # 512 queries/keys per core chunk
DKV = KVH * HD         # 512
KS = D // 128          # 16 contraction slices
NKT = S // 128         # 16 key tiles
SCALE = 1.0 / float(np.sqrt(HD))

_CACHE = {}


def _build_nc():
    import concourse.tile as tile
    from concourse import bacc, mybir
    from contextlib import ExitStack

    f32 = mybir.dt.float32
    bf = mybir.dt.bfloat16
    AF = mybir.ActivationFunctionType

    nc = bacc.Bacc("TRN2", target_bir_lowering=False, debug=False, num_devices=NCORES)

    xt_d = nc.dram_tensor("xt", [128, KS * SQ], bf, kind="ExternalInput")
    wq_d = nc.dram_tensor("wq", [H, 128, KS * 128], bf, kind="ExternalInput")
    wk_d = nc.dram_tensor("wk", [128, KS * DKV], bf, kind="ExternalInput")
    wv_d = nc.dram_tensor("wv", [128, KS * DKV], bf, kind="ExternalInput")
    wo_d = nc.dram_tensor("wo", [4, 128, KS * 512], bf, kind="ExternalInput")
    bq_d = nc.dram_tensor("bq", [128, H], f32, kind="ExternalInput")
    bk_d = nc.dram_tensor("bk", [128, KVH], f32, kind="ExternalInput")
    bv_d = nc.dram_tensor("bv", [128, DKV], f32, kind="ExternalInput")
    cos_d = nc.dram_tensor("cosq", [128, SQ], f32, kind="ExternalInput")
    sin_d = nc.dram_tensor("sinq", [128, SQ], f32, kind="ExternalInput")
    ones_d = nc.dram_tensor("ones", [128, 128], bf, kind="ExternalInput")
    out_d = nc.dram_tensor("out", [SQ, D], f32, kind="ExternalOutput")

    # two half-AllGathers, each carrying 2 kv heads' K and V (0.5MB/rank):
    # rows [0:256] = KT of the 2 heads, rows [256:512] = their V halves.
    sendA = nc.dram_tensor("sendA", [8, 128, 256], bf)
    sendB = nc.dram_tensor("sendB", [8, 128, 256], bf)
    fullA = nc.dram_tensor("fullA", [G * 8, 128, 256], bf)
    fullB = nc.dram_tensor("fullB", [G * 8, 128, 256], bf)
    RG = [[0, 1, 2, 3], [4, 5, 6, 7]]

    with tile.TileContext(nc) as tc, ExitStack() as ctx:
        const = ctx.enter_context(tc.tile_pool(name="const", bufs=1))
        big = ctx.enter_context(tc.tile_pool(name="big", bufs=1))
        wqp = ctx.enter_context(tc.tile_pool(name="wqp", bufs=3))
        wop = ctx.enter_context(tc.tile_pool(name="wop", bufs=2))
        fp = ctx.enter_context(tc.tile_pool(name="fp", bufs=2))
        rp = ctx.enter_context(tc.tile_pool(name="rp", bufs=3))
        ptp = ctx.enter_context(tc.tile_pool(name="ptp", bufs=5))
        outp = ctx.enter_context(tc.tile_pool(name="outp", bufs=2))
        recs = ctx.enter_context(tc.tile_pool(name="recs", bufs=2))
        aup = ctx.enter_context(tc.tile_pool(name="aup", bufs=4))
        dens = ctx.enter_context(tc.tile_pool(name="dens", bufs=2))
        pp_proj = ctx.enter_context(tc.tile_pool(name="pp_proj", bufs=2, space="PSUM"))
        pp_st = ctx.enter_context(tc.tile_pool(name="pp_st", bufs=2, space="PSUM"))
        pp_av = ctx.enter_context(tc.tile_pool(name="pp_av", bufs=2, space="PSUM"))
        pp_den = ctx.enter_context(tc.tile_pool(name="pp_den", bufs=2, space="PSUM"))

        # ---------- loads needed by the K/V path, first ----------
        # split big loads into chunks so they spread across DMA queues
        def chunked_load(dst, src_ap, width, n=4):
            step = width // n
            for j in range(n):
                nc.sync.dma_start(dst[:, j * step:(j + 1) * step], src_ap[:, j * step:(j + 1) * step])

        xt = big.tile([128, KS * SQ], bf)       # [p, ks*SQ + n]: xT d-slices
        chunked_load(xt, xt_d.ap(), KS * SQ, 8)
        wk_sb = big.tile([128, KS * DKV], bf)
        wv_sb = big.tile([128, KS * DKV], bf)
        chunked_load(wk_sb, wk_d.ap(), KS * DKV, 4)
        chunked_load(wv_sb, wv_d.ap(), KS * DKV, 4)
        cos_sb = const.tile([128, SQ], f32)
        sin_sb = const.tile([128, SQ], f32)
        nc.sync.dma_start(cos_sb[:], cos_d.ap())
        nc.sync.dma_start(sin_sb[:], sin_d.ap())
        bk_sb = const.tile([128, KVH], f32)
        bv_sb = const.tile([128, DKV], f32)
        nc.sync.dma_start(bk_sb[:], bk_d.ap())
        nc.sync.dma_start(bv_sb[:], bv_d.ap())

        def rope(ps, bias_col, dst):
            # rotate-half form, all ops full-width and partition-aligned:
            # out = q*[cos;cos] + swap(q)*[-sin;sin] with swap via SBUF DMA.
            qf = fp.tile([128, SQ], f32, tag="f")
            nc.scalar.activation(qf[:], ps[:], AF.Identity, bias=bias_col)
            qsw = fp.tile([128, SQ], f32, tag="fsw")
            nc.sync.dma_start(qsw[0:64, :], qf[64:128, :])
            nc.sync.dma_start(qsw[64:128, :], qf[0:64, :])
            ta = rp.tile([128, SQ], f32, tag="rt")
            nc.vector.tensor_mul(ta[:], qf[:], cos_sb[:])
            tb = rp.tile([128, SQ], f32, tag="rt")
            nc.vector.tensor_mul(tb[:], qsw[:], sin_sb[:])
            nc.vector.tensor_add(dst, ta[:], tb[:])

        # ---------- K/V projection for own chunk, RoPE(K), send ----------
        # order: K heads 0-1 -> V (all) -> AG1 fires early -> K heads 2-3 -> AG2
        kt_own = big.tile([128, KVH * SQ], bf)   # [p=hd, kv*SQ + s]
        v_own = big.tile([128, G * DKV], bf)     # [p=s%128, st*DKV + d]

        def kproj(dt):
            ps = pp_proj.tile([128, SQ], f32, tag="proj", name=f"kps{dt}")
            for ks in range(KS):
                nc.tensor.matmul(
                    ps[:],
                    wk_sb[:, ks * DKV + dt * 128: ks * DKV + (dt + 1) * 128],
                    xt[:, ks * SQ:(ks + 1) * SQ],
                    start=(ks == 0), stop=(ks == KS - 1),
                )
            rope(ps, bk_sb[:, dt:dt + 1], kt_own[:, dt * SQ:(dt + 1) * SQ])

        def kv_sends(pair, send_d, h0):
            # V halves packed as [128,256] blocks; layout is just bytes,
            # unpacked with matching APs on the receive side.
            for hh in range(2):
                for blk in range(2):
                    src = kt_own[:, (h0 + hh) * SQ + blk * 256:(h0 + hh) * SQ + (blk + 1) * 256]
                    nc.sync.dma_start(send_d.ap()[2 * hh + blk], src)
            for st in range(G):
                src = v_own[:, st * DKV + pair * 256: st * DKV + pair * 256 + 256]
                nc.sync.dma_start(send_d.ap()[4 + st], src)

        for dt in (0, 1):
            kproj(dt)
        for st in range(G):
            ps = pp_proj.tile([128, DKV], f32, tag="proj")
            for ks in range(KS):
                nc.tensor.matmul(
                    ps[:],
                    xt[:, ks * SQ + st * 128: ks * SQ + st * 128 + 128],
                    wv_sb[:, ks * DKV:(ks + 1) * DKV],
                    start=(ks == 0), stop=(ks == KS - 1),
                )
            nc.vector.tensor_add(v_own[:, st * DKV:(st + 1) * DKV], ps[:], bv_sb[:])
        kv_sends(0, sendA, 0)
        nc.gpsimd.collective_compute(
            "AllGather", mybir.AluOpType.bypass,
            ins=[sendA.ap()], outs=[fullA.ap()], replica_groups=RG,
        )
        for dt in (2, 3):
            kproj(dt)
        kv_sends(1, sendB, 2)
        nc.gpsimd.collective_compute(
            "AllGather", mybir.AluOpType.bypass,
            ins=[sendB.ap()], outs=[fullB.ap()], replica_groups=RG,
        )

        # ---------- remaining consts ----------
        bq_sb = const.tile([128, H], f32)
        ones_sb = const.tile([128, 128], bf)
        nc.sync.dma_start(bq_sb[:], bq_d.ap())
        nc.sync.dma_start(ones_sb[:], ones_d.ap())

        # ---------- Q projection + RoPE (overlaps AllGather) ----------
        qt_sb = big.tile([128, H * SQ], bf)      # [p=hd, h*SQ + q]
        for ht in range(H):
            wq_t = wqp.tile([128, KS * 128], bf, tag="wq")
            with tc.tile_wait_until(0.010):
                for j in range(4):
                    nc.sync.dma_start(wq_t[:, j * 512:(j + 1) * 512], wq_d.ap()[ht][:, j * 512:(j + 1) * 512])
            ps = pp_proj.tile([128, SQ], f32, tag="proj")
            for ks in range(KS):
                nc.tensor.matmul(
                    ps[:],
                    wq_t[:, ks * 128:(ks + 1) * 128],
                    xt[:, ks * SQ:(ks + 1) * SQ],
                    start=(ks == 0), stop=(ks == KS - 1),
                )
            rope(ps, bq_sb[:, ht:ht + 1], qt_sb[:, ht * SQ:(ht + 1) * SQ])

        # ---------- gather K/V full ----------
        ktf = big.tile([128, KVH * S], bf)       # [p=hd, kv*S + (g*SQ + s)]
        vf = big.tile([128, (G * G) * DKV], bf)  # [p=s%128, (g*4+st)*DKV + d]
        for pair, full_d in enumerate([fullA, fullB]):
            for g in range(G):
                for hh in range(2):
                    h = pair * 2 + hh
                    for blk in range(2):
                        dst = ktf[:, h * S + g * SQ + blk * 256: h * S + g * SQ + (blk + 1) * 256]
                        nc.gpsimd.dma_start(dst, full_d.ap()[g * 8 + 2 * hh + blk])
                for st in range(G):
                    dst = vf[:, (g * G + st) * DKV + pair * 256:(g * G + st) * DKV + pair * 256 + 256]
                    nc.gpsimd.dma_start(dst, full_d.ap()[g * 8 + 4 + st])

        # ---------- attention per head ----------
        a_sb = big.tile([128, H * SQ], bf)       # [p=hd, h*SQ + q]  (AV^T, normalized)
        for h in range(H):
            kv = h // R
            av = pp_av.tile([128, SQ], f32, tag="av")
            den = pp_den.tile([128, SQ], f32, tag="den")
            pts = [None] * NKT

            def av_den(kt):
                nc.tensor.matmul(
                    av[:],
                    vf[:, kt * DKV + kv * 128: kt * DKV + (kv + 1) * 128],
                    pts[kt][:],
                    start=(kt == 0), stop=(kt == NKT - 1),
                )
                nc.tensor.matmul(
                    den[:], ones_sb[:], pts[kt][:],
                    start=(kt == 0), stop=(kt == NKT - 1),
                )

            # software pipeline: AV/den run one k-tile behind scores/exp so
            # the PE never waits on the exp of the tile it just produced.
            for kt in range(NKT):
                st_ps = pp_st.tile([128, SQ], f32, tag="st")
                nc.tensor.matmul(
                    st_ps[:],
                    ktf[:, kv * S + kt * 128: kv * S + (kt + 1) * 128],
                    qt_sb[:, h * SQ:(h + 1) * SQ],
                    start=True, stop=True,
                )
                pt = ptp.tile([128, SQ], bf, tag="pt")
                nc.scalar.activation(pt[:], st_ps[:], AF.Exp, scale=SCALE)
                pts[kt] = pt
                if kt >= 1:
                    av_den(kt - 1)
            av_den(NKT - 1)
            # free the PSUM banks immediately via ScalarE copies so attention
            # never stalls on the DVE backlog; normalize runs asynchronously.
            a_un = aup.tile([128, SQ], bf, tag="aun")
            nc.scalar.activation(a_un[:], av[:], AF.Copy)
            den_sb = dens.tile([128, SQ], f32, tag="densb")
            nc.scalar.activation(den_sb[:], den[:], AF.Copy)
            recb = recs.tile([128, SQ], f32, tag="recb")
            nc.vector.reciprocal_approx_fast(recb[:], den_sb[:])
            nc.vector.tensor_mul(a_sb[:, h * SQ:(h + 1) * SQ], a_un[:], recb[:])

        # ---------- output projection ----------
        for nt in range(4):
            wo_t = wop.tile([128, KS * 512], bf, tag="wo")
            with tc.tile_wait_until(0.200):
                for ct in range(KS):
                    nc.sync.dma_start(wo_t[:, ct * 512:(ct + 1) * 512], wo_d.ap()[nt][:, ct * 512:(ct + 1) * 512])
            for qt in range(4):
                ps = pp_proj.tile([128, 512], f32, tag="proj")
                for ct in range(KS):
                    nc.tensor.matmul(
                        ps[:],
                        a_sb[:, ct * SQ + qt * 128: ct * SQ + qt * 128 + 128],
                        wo_t[:, ct * 512:(ct + 1) * 512],
                        start=(ct == 0), stop=(ct == KS - 1),
                    )
                ot = outp.tile([128, 512], f32, tag="ot")
                nc.scalar.activation(ot[:], ps[:], AF.Copy)
                nc.sync.dma_start(out_d.ap()[qt * 128:(qt + 1) * 128, nt * 512:(nt + 1) * 512], ot[:])

    nc.compile()
    return nc


def get_nc():
    if "nc" not in _CACHE:
        _CACHE["nc"] = _build_nc()
    return _CACHE["nc"]


def make_in_maps(x, wq, bq, wk, bk, wv, bv, wo):
    bf16 = ml_dtypes.bfloat16
    perm = np.concatenate([np.arange(0, HD, 2), np.arange(1, HD, 2)])
    qcols = np.concatenate([h * HD + perm for h in range(H)])
    kcols = np.concatenate([h * HD + perm for h in range(KVH)])
    wq_p = wq[:, qcols]
    bq_p = np.ascontiguousarray(bq[qcols].reshape(H, HD).T).astype(np.float32)
    wk_p = wk[:, kcols]
    bk_p = np.ascontiguousarray(bk[kcols].reshape(KVH, HD).T).astype(np.float32)
    # pretile so every DMA is contiguous: wq [ht][p][ks][c], wk/wv [p][ks][c],
    # wo [nt][p][ct][c]
    wq_t = np.ascontiguousarray(
        wq_p.reshape(KS, 128, H, 128).transpose(2, 1, 0, 3).reshape(H, 128, KS * 128)
    ).astype(bf16)
    wk_t = np.ascontiguousarray(
        wk_p.reshape(KS, 128, DKV).transpose(1, 0, 2).reshape(128, KS * DKV)
    ).astype(bf16)
    wv_t = np.ascontiguousarray(
        wv.reshape(KS, 128, DKV).transpose(1, 0, 2).reshape(128, KS * DKV)
    ).astype(bf16)
    wo_t = np.ascontiguousarray(
        wo.reshape(KS, 128, 4, 512).transpose(2, 1, 0, 3).reshape(4, 128, KS * 512)
    ).astype(bf16)
    bv_rep = np.tile(bv.astype(np.float32), (128, 1))
    theta = (10000.0 ** (-np.arange(64, dtype=np.float64) / 64.0))
    ang = np.outer(np.arange(S, dtype=np.float64), theta)  # [S, 64]
    c = np.cos(ang).T.astype(np.float32)  # [64, S]
    s = np.sin(ang).T.astype(np.float32)
    cosT = np.concatenate([c, c], axis=0)      # [128, S]
    sinT = np.concatenate([-s, s], axis=0)     # [128, S]
    ones = np.ones((128, 128), dtype=bf16)

    in_maps = []
    for b in range(B):
        for g in range(G):
            sl = slice(g * SQ, (g + 1) * SQ)
            xt_c = np.ascontiguousarray(
                x[b, sl, :].T.reshape(KS, 128, SQ).transpose(1, 0, 2).reshape(128, KS * SQ)
            ).astype(bf16)
            in_maps.append({
                "xt": xt_c,
                "wq": wq_t, "wk": wk_t, "wv": wv_t, "wo": wo_t,
                "bq": bq_p, "bk": bk_p, "bv": bv_rep,
                "cosq": np.ascontiguousarray(cosT[:, sl]),
                "sinq": np.ascontiguousarray(sinT[:, sl]),
                "ones": ones,
            })
    return in_maps


def assemble(results):
    out = np.empty((B, S, D), np.float32)
    for b in range(B):
        for g in range(G):
            out[b, g * SQ:(g + 1) * SQ, :] = results[b * G + g]["out"]
    return out


def kernel(x, wq, bq, wk, bk, wv, bv, wo):
    from concourse.bass_utils import run_bass_kernel_spmd

    nc = get_nc()
    in_maps = make_in_maps(x, wq, bq, wk, bk, wv, bv, wo)
    # run twice and return the second result: the first execution after a
    # NEFF load has occasionally produced stale collective output.
    run_bass_kernel_spmd(nc, in_maps, core_ids=list(range(NCORES)))
    res = run_bass_kernel_spmd(nc, in_maps, core_ids=list(range(NCORES)))
    return assemble(res.results)


# revision 49
# speedup vs baseline: 1.0266x; 1.0266x over previous
"""Multi-head GQA attention (B=2, S=2048, D=2048, H=16, KVH=4) on 8 TRN2
NeuronCores.

Sharding: core i = (b, g) with b = i // 4 (batch), g = i % 4 (sequence
chunk of 512 queries). Each core computes Q for its 512 queries over all
16 heads, K/V for its own 512 sequence positions, AllGathers K/V within
its 4-core batch group, then runs full attention + output projection for
its query chunk. Host concatenates the 8 [512, 2048] chunks.

Layout strategy (no on-chip transposes):
 - host passes x transposed per chunk (xT [D, 512]) so projections
   computed as w.T @ xT yield QT/KT with head-dim on partitions —
   exactly the operand layout attention needs.
 - wq/wk columns permuted per head (even dims first, odd second) so RoPE
   halves are contiguous partition ranges [0:64)/[64:128). Scores are
   permutation-invariant since q and k are permuted identically.
 - scores computed transposed (ST[k, q] = KT.T @ QT), exp'd on ScalarE
   straight out of PSUM (scale=1/sqrt(HD) folded in, no max-subtraction:
   scores are O(10) so f32 exp is safe), giving probs in the [k, q]
   layout the AV matmul wants as its moving operand.
 - softmax denominator from an all-ones [k,128] stationary matmul: the
   output is the denominator replicated across all 128 partitions, so
   normalization is reciprocal + elementwise multiply, no broadcast.
 - weights are host-pretiled so every DMA is a contiguous block.
"""

import numpy as np
import ml_dtypes

B, S, D = 2, 2048, 2048
H, KVH = 16, 4
HD = D // H            # 128
R = H // KVH           # 4 (GQA repeat)
NCORES = 8
G = 4                  # cores per batch group = seq chunks
SQ = S // G            # 512 queries/keys per core chunk
DKV = KVH * HD         # 512
KS = D // 128          # 16 contraction slices
NKT = S // 128         # 16 key tiles
SCALE = 1.0 / float(np.sqrt(HD))

_CACHE = {}


def _build_nc():
    import concourse.tile as tile
    from concourse import bacc, mybir
    from contextlib import ExitStack

    f32 = mybir.dt.float32
    bf = mybir.dt.bfloat16
    AF = mybir.ActivationFunctionType

    nc = bacc.Bacc("TRN2", target_bir_lowering=False, debug=False, num_devices=NCORES)

    xt_d = nc.dram_tensor("xt", [128, KS * SQ], bf, kind="ExternalInput")
    wq_d = nc.dram_tensor("wq", [H, 128, KS * 128], bf, kind="ExternalInput")
    wk_d = nc.dram_tensor("wk", [128, KS * DKV], bf, kind="ExternalInput")
    wv_d = nc.dram_tensor("wv", [128, KS * DKV], bf, kind="ExternalInput")
    wo_d = nc.dram_tensor("wo", [4, 128, KS * 512], bf, kind="ExternalInput")
    bq_d = nc.dram_tensor("bq", [128, H], f32, kind="ExternalInput")
    bk_d = nc.dram_tensor("bk", [128, KVH], f32, kind="ExternalInput")
    bv_d = nc.dram_tensor("bv", [128, DKV], f32, kind="ExternalInput")
    cos_d = nc.dram_tensor("cosq", [128, SQ], f32, kind="ExternalInput")
    sin_d = nc.dram_tensor("sinq", [128, SQ], f32, kind="ExternalInput")
    ones_d = nc.dram_tensor("ones", [128, 128], bf, kind="ExternalInput")
    out_d = nc.dram_tensor("out", [SQ, D], f32, kind="ExternalOutput")

    # two half-AllGathers, each carrying 2 kv heads' K and V (0.5MB/rank):
    # rows [0:256] = KT of the 2 heads, rows [256:512] = their V halves.
    sendA = nc.dram_tensor("sendA", [8, 128, 256], bf)
    sendB = nc.dram_tensor("sendB", [8, 128, 256], bf)
    fullA = nc.dram_tensor("fullA", [G * 8, 128, 256], bf)
    fullB = nc.dram_tensor("fullB", [G * 8, 128, 256], bf)
    RG = [[0, 1, 2, 3], [4, 5, 6, 7]]

    with tile.TileContext(nc) as tc, ExitStack() as ctx:
        const = ctx.enter_context(tc.tile_pool(name="const", bufs=1))
        big = ctx.enter_context(tc.tile_pool(name="big", bufs=1))
        wqp = ctx.enter_context(tc.tile_pool(name="wqp", bufs=4))
        wop = ctx.enter_context(tc.tile_pool(name="wop", bufs=2))
        fp = ctx.enter_context(tc.tile_pool(name="fp", bufs=2))
        rp = ctx.enter_context(tc.tile_pool(name="rp", bufs=3))
        ptp = ctx.enter_context(tc.tile_pool(name="ptp", bufs=5))
        outp = ctx.enter_context(tc.tile_pool(name="outp", bufs=2))
        recs = ctx.enter_context(tc.tile_pool(name="recs", bufs=1))
        aup = ctx.enter_context(tc.tile_pool(name="aup", bufs=4))
        dens = ctx.enter_context(tc.tile_pool(name="dens", bufs=2))
        pp_proj = ctx.enter_context(tc.tile_pool(name="pp_proj", bufs=2, space="PSUM"))
        pp_st = ctx.enter_context(tc.tile_pool(name="pp_st", bufs=2, space="PSUM"))
        pp_av = ctx.enter_context(tc.tile_pool(name="pp_av", bufs=2, space="PSUM"))
        pp_den = ctx.enter_context(tc.tile_pool(name="pp_den", bufs=2, space="PSUM"))

        # ---------- loads needed by the K/V path, first ----------
        # split big loads into chunks so they spread across DMA queues
        def chunked_load(dst, src_ap, width, n=4):
            step = width // n
            for j in range(n):
                nc.sync.dma_start(dst[:, j * step:(j + 1) * step], src_ap[:, j * step:(j + 1) * step])

        xt = big.tile([128, KS * SQ], bf)       # [p, ks*SQ + n]: xT d-slices
        chunked_load(xt, xt_d.ap(), KS * SQ, 8)
        wk_sb = big.tile([128, KS * DKV], bf)
        wv_sb = big.tile([128, KS * DKV], bf)
        chunked_load(wk_sb, wk_d.ap(), KS * DKV, 4)
        chunked_load(wv_sb, wv_d.ap(), KS * DKV, 4)
        cos_sb = const.tile([128, SQ], f32)
        sin_sb = const.tile([128, SQ], f32)
        nc.sync.dma_start(cos_sb[:], cos_d.ap())
        nc.sync.dma_start(sin_sb[:], sin_d.ap())
        bk_sb = const.tile([128, KVH], f32)
        bv_sb = const.tile([128, DKV], f32)
        nc.sync.dma_start(bk_sb[:], bk_d.ap())
        nc.sync.dma_start(bv_sb[:], bv_d.ap())

        def rope(ps, bias_col, dst):
            # rotate-half form, all ops full-width and partition-aligned:
            # out = q*[cos;cos] + swap(q)*[-sin;sin] with swap via SBUF DMA.
            qf = fp.tile([128, SQ], f32, tag="f")
            nc.scalar.activation(qf[:], ps[:], AF.Identity, bias=bias_col)
            qsw = fp.tile([128, SQ], f32, tag="fsw")
            nc.sync.dma_start(qsw[0:64, :], qf[64:128, :])
            nc.sync.dma_start(qsw[64:128, :], qf[0:64, :])
            ta = rp.tile([128, SQ], f32, tag="rt")
            nc.vector.tensor_mul(ta[:], qf[:], cos_sb[:])
            tb = rp.tile([128, SQ], f32, tag="rt")
            nc.vector.tensor_mul(tb[:], qsw[:], sin_sb[:])
            nc.vector.tensor_add(dst, ta[:], tb[:])

        # ---------- K/V projection for own chunk, RoPE(K), send ----------
        # order: K heads 0-1 -> V (all) -> AG1 fires early -> K heads 2-3 -> AG2
        kt_own = big.tile([128, KVH * SQ], bf)   # [p=hd, kv*SQ + s]
        v_own = big.tile([128, G * DKV], bf)     # [p=s%128, st*DKV + d]

        def kproj(dt):
            ps = pp_proj.tile([128, SQ], f32, tag="proj", name=f"kps{dt}")
            for ks in range(KS):
                nc.tensor.matmul(
                    ps[:],
                    wk_sb[:, ks * DKV + dt * 128: ks * DKV + (dt + 1) * 128],
                    xt[:, ks * SQ:(ks + 1) * SQ],
                    start=(ks == 0), stop=(ks == KS - 1),
                )
            rope(ps, bk_sb[:, dt:dt + 1], kt_own[:, dt * SQ:(dt + 1) * SQ])

        def kv_sends(pair, send_d, h0):
            # V halves packed as [128,256] blocks; layout is just bytes,
            # unpacked with matching APs on the receive side.
            for hh in range(2):
                for blk in range(2):
                    src = kt_own[:, (h0 + hh) * SQ + blk * 256:(h0 + hh) * SQ + (blk + 1) * 256]
                    nc.sync.dma_start(send_d.ap()[2 * hh + blk], src)
            for st in range(G):
                src = v_own[:, st * DKV + pair * 256: st * DKV + pair * 256 + 256]
                nc.sync.dma_start(send_d.ap()[4 + st], src)

        for dt in (0, 1):
            kproj(dt)
        for st in range(G):
            ps = pp_proj.tile([128, DKV], f32, tag="proj")
            for ks in range(KS):
                nc.tensor.matmul(
                    ps[:],
                    xt[:, ks * SQ + st * 128: ks * SQ + st * 128 + 128],
                    wv_sb[:, ks * DKV:(ks + 1) * DKV],
                    start=(ks == 0), stop=(ks == KS - 1),
                )
            nc.vector.tensor_add(v_own[:, st * DKV:(st + 1) * DKV], ps[:], bv_sb[:])
        kv_sends(0, sendA, 0)
        nc.gpsimd.collective_compute(
            "AllGather", mybir.AluOpType.bypass,
            ins=[sendA.ap()], outs=[fullA.ap()], replica_groups=RG,
        )
        for dt in (2, 3):
            kproj(dt)
        kv_sends(1, sendB, 2)
        nc.gpsimd.collective_compute(
            "AllGather", mybir.AluOpType.bypass,
            ins=[sendB.ap()], outs=[fullB.ap()], replica_groups=RG,
        )

        # ---------- remaining consts ----------
        bq_sb = const.tile([128, H], f32)
        ones_sb = const.tile([128, 128], bf)
        nc.sync.dma_start(bq_sb[:], bq_d.ap())
        nc.sync.dma_start(ones_sb[:], ones_d.ap())

        # ---------- Q projection + RoPE (overlaps AllGather) ----------
        qt_sb = big.tile([128, H * SQ], bf)      # [p=hd, h*SQ + q]
        for ht in range(H):
            wq_t = wqp.tile([128, KS * 128], bf, tag="wq")
            with tc.tile_wait_until(0.010):
                for j in range(4):
                    nc.sync.dma_start(wq_t[:, j * 512:(j + 1) * 512], wq_d.ap()[ht][:, j * 512:(j + 1) * 512])
            ps = pp_proj.tile([128, SQ], f32, tag="proj")
            for ks in range(KS):
                nc.tensor.matmul(
                    ps[:],
                    wq_t[:, ks * 128:(ks + 1) * 128],
                    xt[:, ks * SQ:(ks + 1) * SQ],
                    start=(ks == 0), stop=(ks == KS - 1),
                )
            rope(ps, bq_sb[:, ht:ht + 1], qt_sb[:, ht * SQ:(ht + 1) * SQ])

        # ---------- gather K/V full ----------
        # separate tiles per AG pair so heads 0-7 aren't gated on AG2
        # (dependency tracking is tile-granular).
        ktfp = [big.tile([128, 2 * S], bf, name=f"ktf{p}") for p in range(2)]
        vfp = [big.tile([128, (G * G) * 256], bf, name=f"vf{p}") for p in range(2)]
        for pair, full_d in enumerate([fullA, fullB]):
            ktf_t, vf_t = ktfp[pair], vfp[pair]
            for g in range(G):
                for hh in range(2):
                    for blk in range(2):
                        dst = ktf_t[:, hh * S + g * SQ + blk * 256: hh * S + g * SQ + (blk + 1) * 256]
                        nc.gpsimd.dma_start(dst, full_d.ap()[g * 8 + 2 * hh + blk])
                for st in range(G):
                    dst = vf_t[:, (g * G + st) * 256:(g * G + st) * 256 + 256]
                    nc.gpsimd.dma_start(dst, full_d.ap()[g * 8 + 4 + st])

        # ---------- attention per head ----------
        a_sb = big.tile([128, H * SQ], bf)       # [p=hd, h*SQ + q]  (AV^T, normalized)
        for h in range(H):
            kv = h // R
            ktf_t, vf_t = ktfp[kv // 2], vfp[kv // 2]
            kvh = kv % 2
            av = pp_av.tile([128, SQ], f32, tag="av")
            den = pp_den.tile([128, SQ], f32, tag="den")
            pts = [None] * NKT

            def av_den(kt):
                nc.tensor.matmul(
                    av[:],
                    vf_t[:, kt * 256 + kvh * 128: kt * 256 + (kvh + 1) * 128],
                    pts[kt][:],
                    start=(kt == 0), stop=(kt == NKT - 1),
                )
                nc.tensor.matmul(
                    den[:], ones_sb[:], pts[kt][:],
                    start=(kt == 0), stop=(kt == NKT - 1),
                )

            # software pipeline: AV/den run one k-tile behind scores/exp so
            # the PE never waits on the exp of the tile it just produced.
            for kt in range(NKT):
                st_ps = pp_st.tile([128, SQ], f32, tag="st")
                nc.tensor.matmul(
                    st_ps[:],
                    ktf_t[:, kvh * S + kt * 128: kvh * S + (kt + 1) * 128],
                    qt_sb[:, h * SQ:(h + 1) * SQ],
                    start=True, stop=True,
                )
                pt = ptp.tile([128, SQ], bf, tag="pt")
                nc.scalar.activation(pt[:], st_ps[:], AF.Exp, scale=SCALE)
                pts[kt] = pt
                if kt >= 1:
                    av_den(kt - 1)
            av_den(NKT - 1)
            # free the PSUM banks immediately via ScalarE copies so attention
            # never stalls on the DVE backlog; normalize runs asynchronously.
            a_un = aup.tile([128, SQ], bf, tag="aun")
            nc.scalar.activation(a_un[:], av[:], AF.Copy)
            den_sb = dens.tile([128, SQ], f32, tag="densb")
            nc.scalar.activation(den_sb[:], den[:], AF.Copy)
            recb = recs.tile([128, SQ], f32, tag="recb")
            nc.vector.reciprocal_approx_fast(recb[:], den_sb[:])
            nc.vector.tensor_mul(a_sb[:, h * SQ:(h + 1) * SQ], a_un[:], recb[:])

        # ---------- output projection ----------
        for nt in range(4):
            wo_t = wop.tile([128, KS * 512], bf, tag="wo")
            with tc.tile_wait_until(0.200):
                for ct in range(KS):
                    nc.sync.dma_start(wo_t[:, ct * 512:(ct + 1) * 512], wo_d.ap()[nt][:, ct * 512:(ct + 1) * 512])
            for qt in range(4):
                ps = pp_proj.tile([128, 512], f32, tag="proj")
                for ct in range(KS):
                    nc.tensor.matmul(
                        ps[:],
                        a_sb[:, ct * SQ + qt * 128: ct * SQ + qt * 128 + 128],
                        wo_t[:, ct * 512:(ct + 1) * 512],
                        start=(ct == 0), stop=(ct == KS - 1),
                    )
                ot = outp.tile([128, 512], f32, tag="ot")
                nc.scalar.activation(ot[:], ps[:], AF.Copy)
                nc.sync.dma_start(out_d.ap()[qt * 128:(qt + 1) * 128, nt * 512:(nt + 1) * 512], ot[:])

    nc.compile()
    return nc


def get_nc():
    if "nc" not in _CACHE:
        _CACHE["nc"] = _build_nc()
    return _CACHE["nc"]


def make_in_maps(x, wq, bq, wk, bk, wv, bv, wo):
    bf16 = ml_dtypes.bfloat16
    perm = np.concatenate([np.arange(0, HD, 2), np.arange(1, HD, 2)])
    qcols = np.concatenate([h * HD + perm for h in range(H)])
    kcols = np.concatenate([h * HD + perm for h in range(KVH)])
    wq_p = wq[:, qcols]
    bq_p = np.ascontiguousarray(bq[qcols].reshape(H, HD).T).astype(np.float32)
    wk_p = wk[:, kcols]
    bk_p = np.ascontiguousarray(bk[kcols].reshape(KVH, HD).T).astype(np.float32)
    # pretile so every DMA is contiguous: wq [ht][p][ks][c], wk/wv [p][ks][c],
    # wo [nt][p][ct][c]
    wq_t = np.ascontiguousarray(
        wq_p.reshape(KS, 128, H, 128).transpose(2, 1, 0, 3).reshape(H, 128, KS * 128)
    ).astype(bf16)
    wk_t = np.ascontiguousarray(
        wk_p.reshape(KS, 128, DKV).transpose(1, 0, 2).reshape(128, KS * DKV)
    ).astype(bf16)
    wv_t = np.ascontiguousarray(
        wv.reshape(KS, 128, DKV).transpose(1, 0, 2).reshape(128, KS * DKV)
    ).astype(bf16)
    wo_t = np.ascontiguousarray(
        wo.reshape(KS, 128, 4, 512).transpose(2, 1, 0, 3).reshape(4, 128, KS * 512)
    ).astype(bf16)
    bv_rep = np.tile(bv.astype(np.float32), (128, 1))
    theta = (10000.0 ** (-np.arange(64, dtype=np.float64) / 64.0))
    ang = np.outer(np.arange(S, dtype=np.float64), theta)  # [S, 64]
    c = np.cos(ang).T.astype(np.float32)  # [64, S]
    s = np.sin(ang).T.astype(np.float32)
    cosT = np.concatenate([c, c], axis=0)      # [128, S]
    sinT = np.concatenate([-s, s], axis=0)     # [128, S]
    ones = np.ones((128, 128), dtype=bf16)

    in_maps = []
    for b in range(B):
        for g in range(G):
            sl = slice(g * SQ, (g + 1) * SQ)
            xt_c = np.ascontiguousarray(
                x[b, sl, :].T.reshape(KS, 128, SQ).transpose(1, 0, 2).reshape(128, KS * SQ)
            ).astype(bf16)
            in_maps.append({
                "xt": xt_c,
                "wq": wq_t, "wk": wk_t, "wv": wv_t, "wo": wo_t,
                "bq": bq_p, "bk": bk_p, "bv": bv_rep,
                "cosq": np.ascontiguousarray(cosT[:, sl]),
                "sinq": np.ascontiguousarray(sinT[:, sl]),
                "ones": ones,
            })
    return in_maps


def assemble(results):
    out = np.empty((B, S, D), np.float32)
    for b in range(B):
        for g in range(G):
            out[b, g * SQ:(g + 1) * SQ, :] = results[b * G + g]["out"]
    return out


def kernel(x, wq, bq, wk, bk, wv, bv, wo):
    from concourse.bass_utils import run_bass_kernel_spmd

    nc = get_nc()
    in_maps = make_in_maps(x, wq, bq, wk, bk, wv, bv, wo)
    # run twice and return the second result: the first execution after a
    # NEFF load has occasionally produced stale collective output.
    run_bass_kernel_spmd(nc, in_maps, core_ids=list(range(NCORES)))
    res = run_bass_kernel_spmd(nc, in_maps, core_ids=list(range(NCORES)))
    return assemble(res.results)


# revision 55
# speedup vs baseline: 1.0614x; 1.0338x over previous
"""Multi-head GQA attention (B=2, S=2048, D=2048, H=16, KVH=4) on 8 TRN2
NeuronCores.

Sharding: core i = (b, g) with b = i // 4 (batch), g = i % 4 (sequence
chunk of 512 queries). Each core computes Q for its 512 queries over all
16 heads, K/V for its own 512 sequence positions, AllGathers K/V within
its 4-core batch group, then runs full attention + output projection for
its query chunk. Host concatenates the 8 [512, 2048] chunks.

Layout strategy (no on-chip transposes):
 - host passes x transposed per chunk (xT [D, 512]) so projections
   computed as w.T @ xT yield QT/KT with head-dim on partitions —
   exactly the operand layout attention needs.
 - wq/wk columns permuted per head (even dims first, odd second) so RoPE
   halves are contiguous partition ranges [0:64)/[64:128). Scores are
   permutation-invariant since q and k are permuted identically.
 - scores computed transposed (ST[k, q] = KT.T @ QT), exp'd on ScalarE
   straight out of PSUM (scale=1/sqrt(HD) folded in, no max-subtraction:
   scores are O(10) so f32 exp is safe), giving probs in the [k, q]
   layout the AV matmul wants as its moving operand.
 - softmax denominator from an all-ones [k,128] stationary matmul: the
   output is the denominator replicated across all 128 partitions, so
   normalization is reciprocal + elementwise multiply, no broadcast.
 - weights are host-pretiled so every DMA is a contiguous block.
"""

import numpy as np
import ml_dtypes

B, S, D = 2, 2048, 2048
H, KVH = 16, 4
HD = D // H            # 128
R = H // KVH           # 4 (GQA repeat)
NCORES = 8
G = 4                  # cores per batch group = seq chunks
SQ = S // G            # 512 queries/keys per core chunk
DKV = KVH * HD         # 512
KS = D // 128          # 16 contraction slices
NKT = S // 128         # 16 key tiles
SCALE = 1.0 / float(np.sqrt(HD))

_CACHE = {}


def _build_nc():
    import concourse.tile as tile
    from concourse import bacc, mybir
    from contextlib import ExitStack

    f32 = mybir.dt.float32
    bf = mybir.dt.bfloat16
    AF = mybir.ActivationFunctionType

    nc = bacc.Bacc("TRN2", target_bir_lowering=False, debug=False, num_devices=NCORES)

    xt_d = nc.dram_tensor("xt", [128, KS * SQ], bf, kind="ExternalInput")
    wq_d = nc.dram_tensor("wq", [H, 128, KS * 128], bf, kind="ExternalInput")
    wk_d = nc.dram_tensor("wk", [128, KS * DKV], bf, kind="ExternalInput")
    wv_d = nc.dram_tensor("wv", [128, KS * DKV], bf, kind="ExternalInput")
    wo_d = nc.dram_tensor("wo", [4, 128, KS * 512], bf, kind="ExternalInput")
    bq_d = nc.dram_tensor("bq", [128, H], f32, kind="ExternalInput")
    bk_d = nc.dram_tensor("bk", [128, KVH], f32, kind="ExternalInput")
    bv_d = nc.dram_tensor("bv", [128, DKV], f32, kind="ExternalInput")
    cos_d = nc.dram_tensor("cosq", [128, SQ], bf, kind="ExternalInput")
    sin_d = nc.dram_tensor("sinq", [128, SQ], bf, kind="ExternalInput")
    ones_d = nc.dram_tensor("ones", [128, 128], bf, kind="ExternalInput")
    out_d = nc.dram_tensor("out", [SQ, D], f32, kind="ExternalOutput")

    # two half-AllGathers, each carrying 2 kv heads' K and V (0.5MB/rank):
    # rows [0:256] = KT of the 2 heads, rows [256:512] = their V halves.
    sendA = nc.dram_tensor("sendA", [8, 128, 256], bf)
    sendB = nc.dram_tensor("sendB", [8, 128, 256], bf)
    fullA = nc.dram_tensor("fullA", [G * 8, 128, 256], bf)
    fullB = nc.dram_tensor("fullB", [G * 8, 128, 256], bf)
    RG = [[0, 1, 2, 3], [4, 5, 6, 7]]

    with tile.TileContext(nc) as tc, ExitStack() as ctx:
        const = ctx.enter_context(tc.tile_pool(name="const", bufs=1))
        big = ctx.enter_context(tc.tile_pool(name="big", bufs=1))
        wqp = ctx.enter_context(tc.tile_pool(name="wqp", bufs=4))
        wop = ctx.enter_context(tc.tile_pool(name="wop", bufs=2))
        fp = ctx.enter_context(tc.tile_pool(name="fp", bufs=2))
        rp = ctx.enter_context(tc.tile_pool(name="rp", bufs=3))
        ptp = ctx.enter_context(tc.tile_pool(name="ptp", bufs=5))
        outp = ctx.enter_context(tc.tile_pool(name="outp", bufs=2))
        recs = ctx.enter_context(tc.tile_pool(name="recs", bufs=2))
        pp_proj = ctx.enter_context(tc.tile_pool(name="pp_proj", bufs=2, space="PSUM"))
        pp_st = ctx.enter_context(tc.tile_pool(name="pp_st", bufs=2, space="PSUM"))
        pp_av = ctx.enter_context(tc.tile_pool(name="pp_av", bufs=2, space="PSUM"))
        pp_den = ctx.enter_context(tc.tile_pool(name="pp_den", bufs=2, space="PSUM"))

        # ---------- loads needed by the K/V path, first ----------
        # split big loads into chunks so they spread across DMA queues
        def chunked_load(dst, src_ap, width, n=4):
            step = width // n
            for j in range(n):
                nc.sync.dma_start(dst[:, j * step:(j + 1) * step], src_ap[:, j * step:(j + 1) * step])

        xt = big.tile([128, KS * SQ], bf)       # [p, ks*SQ + n]: xT d-slices
        chunked_load(xt, xt_d.ap(), KS * SQ, 8)
        wk_sb = big.tile([128, KS * DKV], bf)
        wv_sb = big.tile([128, KS * DKV], bf)
        chunked_load(wk_sb, wk_d.ap(), KS * DKV, 4)
        chunked_load(wv_sb, wv_d.ap(), KS * DKV, 4)
        cos_sb = const.tile([128, SQ], bf)
        sin_sb = const.tile([128, SQ], bf)
        nc.sync.dma_start(cos_sb[:], cos_d.ap())
        nc.sync.dma_start(sin_sb[:], sin_d.ap())
        bk_sb = const.tile([128, KVH], f32)
        bv_sb = const.tile([128, DKV], f32)
        nc.sync.dma_start(bk_sb[:], bk_d.ap())
        nc.sync.dma_start(bv_sb[:], bv_d.ap())

        def rope(ps, bias_col, dst):
            # rotate-half form, all ops full-width and partition-aligned:
            # out = q*[cos;cos] + swap(q)*[-sin;sin] with swap via SBUF DMA.
            # bf16 throughout for the 2x DVE mode.
            qf = fp.tile([128, SQ], bf, tag="f")
            nc.scalar.activation(qf[:], ps[:], AF.Identity, bias=bias_col)
            qsw = fp.tile([128, SQ], bf, tag="fsw")
            nc.sync.dma_start(qsw[0:64, :], qf[64:128, :])
            nc.sync.dma_start(qsw[64:128, :], qf[0:64, :])
            ta = rp.tile([128, SQ], bf, tag="rt")
            nc.vector.tensor_mul(ta[:], qf[:], cos_sb[:])
            tb = rp.tile([128, SQ], bf, tag="rt")
            nc.vector.tensor_mul(tb[:], qsw[:], sin_sb[:])
            nc.vector.tensor_add(dst, ta[:], tb[:])

        # ---------- K/V projection for own chunk, RoPE(K), send ----------
        # order: K heads 0-1 -> V (all) -> AG1 fires early -> K heads 2-3 -> AG2
        kt_own = big.tile([128, KVH * SQ], bf)   # [p=hd, kv*SQ + s]
        v_own = big.tile([128, G * DKV], bf)     # [p=s%128, st*DKV + d]

        def kproj(dt):
            ps = pp_proj.tile([128, SQ], f32, tag="proj", name=f"kps{dt}")
            for ks in range(KS):
                nc.tensor.matmul(
                    ps[:],
                    wk_sb[:, ks * DKV + dt * 128: ks * DKV + (dt + 1) * 128],
                    xt[:, ks * SQ:(ks + 1) * SQ],
                    start=(ks == 0), stop=(ks == KS - 1),
                )
            rope(ps, bk_sb[:, dt:dt + 1], kt_own[:, dt * SQ:(dt + 1) * SQ])

        def kv_sends(pair, send_d, h0):
            # V halves packed as [128,256] blocks; layout is just bytes,
            # unpacked with matching APs on the receive side.
            for hh in range(2):
                for blk in range(2):
                    src = kt_own[:, (h0 + hh) * SQ + blk * 256:(h0 + hh) * SQ + (blk + 1) * 256]
                    nc.sync.dma_start(send_d.ap()[2 * hh + blk], src)
            for st in range(G):
                src = v_own[:, st * DKV + pair * 256: st * DKV + pair * 256 + 256]
                nc.sync.dma_start(send_d.ap()[4 + st], src)

        for dt in (0, 1):
            kproj(dt)
        for st in range(G):
            ps = pp_proj.tile([128, DKV], f32, tag="proj")
            for ks in range(KS):
                nc.tensor.matmul(
                    ps[:],
                    xt[:, ks * SQ + st * 128: ks * SQ + st * 128 + 128],
                    wv_sb[:, ks * DKV:(ks + 1) * DKV],
                    start=(ks == 0), stop=(ks == KS - 1),
                )
            nc.vector.tensor_add(v_own[:, st * DKV:(st + 1) * DKV], ps[:], bv_sb[:])
        kv_sends(0, sendA, 0)
        nc.gpsimd.collective_compute(
            "AllGather", mybir.AluOpType.bypass,
            ins=[sendA.ap()], outs=[fullA.ap()], replica_groups=RG,
        )
        for dt in (2, 3):
            kproj(dt)
        kv_sends(1, sendB, 2)
        nc.gpsimd.collective_compute(
            "AllGather", mybir.AluOpType.bypass,
            ins=[sendB.ap()], outs=[fullB.ap()], replica_groups=RG,
        )

        # ---------- remaining consts ----------
        bq_sb = const.tile([128, H], f32)
        ones_sb = const.tile([128, 128], bf)
        nc.sync.dma_start(bq_sb[:], bq_d.ap())
        nc.sync.dma_start(ones_sb[:], ones_d.ap())

        # ---------- Q projection + RoPE (overlaps AllGather) ----------
        qt_sb = big.tile([128, H * SQ], bf)      # [p=hd, h*SQ + q]
        for ht in range(H):
            wq_t = wqp.tile([128, KS * 128], bf, tag="wq")
            with tc.tile_wait_until(0.010):
                for j in range(4):
                    nc.sync.dma_start(wq_t[:, j * 512:(j + 1) * 512], wq_d.ap()[ht][:, j * 512:(j + 1) * 512])
            ps = pp_proj.tile([128, SQ], f32, tag="proj")
            for ks in range(KS):
                nc.tensor.matmul(
                    ps[:],
                    wq_t[:, ks * 128:(ks + 1) * 128],
                    xt[:, ks * SQ:(ks + 1) * SQ],
                    start=(ks == 0), stop=(ks == KS - 1),
                )
            rope(ps, bq_sb[:, ht:ht + 1], qt_sb[:, ht * SQ:(ht + 1) * SQ])

        # ---------- gather K/V full ----------
        # separate tiles per AG pair so heads 0-7 aren't gated on AG2
        # (dependency tracking is tile-granular).
        ktfp = [big.tile([128, 2 * S], bf, name=f"ktf{p}") for p in range(2)]
        vfp = [big.tile([128, (G * G) * 256], bf, name=f"vf{p}") for p in range(2)]
        for pair, full_d in enumerate([fullA, fullB]):
            ktf_t, vf_t = ktfp[pair], vfp[pair]
            for g in range(G):
                for hh in range(2):
                    for blk in range(2):
                        dst = ktf_t[:, hh * S + g * SQ + blk * 256: hh * S + g * SQ + (blk + 1) * 256]
                        nc.gpsimd.dma_start(dst, full_d.ap()[g * 8 + 2 * hh + blk])
                for st in range(G):
                    dst = vf_t[:, (g * G + st) * 256:(g * G + st) * 256 + 256]
                    nc.gpsimd.dma_start(dst, full_d.ap()[g * 8 + 4 + st])

        # ---------- attention per head ----------
        a_sb = big.tile([128, H * SQ], bf)       # [p=hd, h*SQ + q]  (AV^T, normalized)
        for h in range(H):
            kv = h // R
            ktf_t, vf_t = ktfp[kv // 2], vfp[kv // 2]
            kvh = kv % 2
            av = pp_av.tile([128, SQ], f32, tag="av")
            den = pp_den.tile([128, SQ], f32, tag="den")
            pts = [None] * NKT

            def av_den(kt):
                nc.tensor.matmul(
                    av[:],
                    vf_t[:, kt * 256 + kvh * 128: kt * 256 + (kvh + 1) * 128],
                    pts[kt][:],
                    start=(kt == 0), stop=(kt == NKT - 1),
                )
                nc.tensor.matmul(
                    den[:], ones_sb[:], pts[kt][:],
                    start=(kt == 0), stop=(kt == NKT - 1),
                )

            # software pipeline: AV/den run one k-tile behind scores/exp so
            # the PE never waits on the exp of the tile it just produced.
            for kt in range(NKT):
                st_ps = pp_st.tile([128, SQ], f32, tag="st")
                nc.tensor.matmul(
                    st_ps[:],
                    ktf_t[:, kvh * S + kt * 128: kvh * S + (kt + 1) * 128],
                    qt_sb[:, h * SQ:(h + 1) * SQ],
                    start=True, stop=True,
                )
                pt = ptp.tile([128, SQ], bf, tag="pt")
                nc.scalar.activation(pt[:], st_ps[:], AF.Exp, scale=SCALE)
                pts[kt] = pt
                if kt >= 1:
                    av_den(kt - 1)
            av_den(NKT - 1)
            recb = recs.tile([128, SQ], f32, tag="recb")
            nc.vector.reciprocal_approx_fast(recb[:], den[:])
            nc.vector.tensor_mul(a_sb[:, h * SQ:(h + 1) * SQ], av[:], recb[:])

        # ---------- output projection ----------
        for nt in range(4):
            wo_t = wop.tile([128, KS * 512], bf, tag="wo")
            with tc.tile_wait_until(0.200):
                for ct in range(KS):
                    nc.sync.dma_start(wo_t[:, ct * 512:(ct + 1) * 512], wo_d.ap()[nt][:, ct * 512:(ct + 1) * 512])
            for qt in range(4):
                ps = pp_proj.tile([128, 512], f32, tag="proj")
                for ct in range(KS):
                    nc.tensor.matmul(
                        ps[:],
                        a_sb[:, ct * SQ + qt * 128: ct * SQ + qt * 128 + 128],
                        wo_t[:, ct * 512:(ct + 1) * 512],
                        start=(ct == 0), stop=(ct == KS - 1),
                    )
                ot = outp.tile([128, 512], f32, tag="ot")
                nc.scalar.activation(ot[:], ps[:], AF.Copy)
                nc.sync.dma_start(out_d.ap()[qt * 128:(qt + 1) * 128, nt * 512:(nt + 1) * 512], ot[:])

    nc.compile()
    return nc


def get_nc():
    if "nc" not in _CACHE:
        _CACHE["nc"] = _build_nc()
    return _CACHE["nc"]


def make_in_maps(x, wq, bq, wk, bk, wv, bv, wo):
    bf16 = ml_dtypes.bfloat16
    perm = np.concatenate([np.arange(0, HD, 2), np.arange(1, HD, 2)])
    qcols = np.concatenate([h * HD + perm for h in range(H)])
    kcols = np.concatenate([h * HD + perm for h in range(KVH)])
    wq_p = wq[:, qcols]
    bq_p = np.ascontiguousarray(bq[qcols].reshape(H, HD).T).astype(np.float32)
    wk_p = wk[:, kcols]
    bk_p = np.ascontiguousarray(bk[kcols].reshape(KVH, HD).T).astype(np.float32)
    # pretile so every DMA is contiguous: wq [ht][p][ks][c], wk/wv [p][ks][c],
    # wo [nt][p][ct][c]
    wq_t = np.ascontiguousarray(
        wq_p.reshape(KS, 128, H, 128).transpose(2, 1, 0, 3).reshape(H, 128, KS * 128)
    ).astype(bf16)
    wk_t = np.ascontiguousarray(
        wk_p.reshape(KS, 128, DKV).transpose(1, 0, 2).reshape(128, KS * DKV)
    ).astype(bf16)
    wv_t = np.ascontiguousarray(
        wv.reshape(KS, 128, DKV).transpose(1, 0, 2).reshape(128, KS * DKV)
    ).astype(bf16)
    wo_t = np.ascontiguousarray(
        wo.reshape(KS, 128, 4, 512).transpose(2, 1, 0, 3).reshape(4, 128, KS * 512)
    ).astype(bf16)
    bv_rep = np.tile(bv.astype(np.float32), (128, 1))
    theta = (10000.0 ** (-np.arange(64, dtype=np.float64) / 64.0))
    ang = np.outer(np.arange(S, dtype=np.float64), theta)  # [S, 64]
    c = np.cos(ang).T.astype(np.float32)  # [64, S]
    s = np.sin(ang).T.astype(np.float32)
    cosT = np.concatenate([c, c], axis=0)      # [128, S]
    sinT = np.concatenate([-s, s], axis=0)     # [128, S]
    ones = np.ones((128, 128), dtype=bf16)

    in_maps = []
    for b in range(B):
        for g in range(G):
            sl = slice(g * SQ, (g + 1) * SQ)
            xt_c = np.ascontiguousarray(
                x[b, sl, :].T.reshape(KS, 128, SQ).transpose(1, 0, 2).reshape(128, KS * SQ)
            ).astype(bf16)
            in_maps.append({
                "xt": xt_c,
                "wq": wq_t, "wk": wk_t, "wv": wv_t, "wo": wo_t,
                "bq": bq_p, "bk": bk_p, "bv": bv_rep,
                "cosq": np.ascontiguousarray(cosT[:, sl]).astype(bf16),
                "sinq": np.ascontiguousarray(sinT[:, sl]).astype(bf16),
                "ones": ones,
            })
    return in_maps


def assemble(results):
    out = np.empty((B, S, D), np.float32)
    for b in range(B):
        for g in range(G):
            out[b, g * SQ:(g + 1) * SQ, :] = results[b * G + g]["out"]
    return out


def kernel(x, wq, bq, wk, bk, wv, bv, wo):
    from concourse.bass_utils import run_bass_kernel_spmd

    nc = get_nc()
    in_maps = make_in_maps(x, wq, bq, wk, bk, wv, bv, wo)
    # run twice and return the second result: the first execution after a
    # NEFF load has occasionally produced stale collective output.
    run_bass_kernel_spmd(nc, in_maps, core_ids=list(range(NCORES)))
    res = run_bass_kernel_spmd(nc, in_maps, core_ids=list(range(NCORES)))
    return assemble(res.results)
